# revision 35
# baseline (speedup 1.0000x reference)
"""ComplexRNN Trainium2 kernel.

10-layer tanh RNN, B=1024, T=512, D=16, H=30, final FC on last timestep.

Strategy (per core, 8-way batch-parallel, 128 batch rows/core):
  - Hidden-major layout: state h^l lives in SBUF as [30 partitions, 128 batch].
  - Layer wavefront: at step s, layer l computes timestep t = s - l. All
    10 layers advance each step; all dependencies are on step s-1.
  - States packed into 3 "region" windows of 128 partitions (4 slots of 32):
      W1 = [h0 h1 h2 h3], W2 = [h3' h4 h5 h6], W3 = [h6' h7 h8 h9]
    (h3', h6' are duplicates written by extra matmuls so each layer finds
    its feed + recurrent state inside one 128-partition window).
  - ONE fp16 matmul per region per step (K=128, M=128, N=128): a block
    matrix computes all of the region's layer updates at once. Plus one
    "inject" matmul for x_t @ W_ih0^T (reading a host-pre-transposed x
    ring) and two duplicate matmuls (dup3 -> h3', dup6 -> h6'), whose
    weights are column slices of the merged region matrices.
  - Per-slot ones rows: slot j's ones lives at partition 32j+30; its
    bias rides on that row, and each ones regenerates from the previous
    slot's ones (weight 30.0 -> tanh -> 1.0), forming a delay chain
    seeded only at W1-slot0. A slot's ones (and hence its bias) turns on
    exactly when its layer becomes active, so inactive slots stay
    exactly 0 through the wavefront warmup: no warmup zeroing at all.
  - tanh: ACTIVATE (PSUM -> SBUF fp16); ScalarE is the critical engine.
    THREE acts per step (one per 128-col region window). With 2 acts the
    binding cycle is act(473ns) + its 3-matmul dependency chain (442ns)
    = 915ns/step; with 3 acts each act gates only 1-2 matmuls (chain
    ~390ns hides under the other two acts) and the period drops to the
    ScalarE issue floor ~3*258 = 775ns/step.
  - Each window has its OWN SBUF state tile and PSUM tile: dependency
    tracking is tile-granular, so a shared [128, 2*BC] tile gives act1 a
    false RAW dep on R1 and inject a false WAR on act2, serializing the
    step (and collapsing the PE to its cold clock).
  - The PE executes in order, so matmuls are emitted monotone in their
    gating act (R0,dup3 | R1,dup6 | R2) and inject(s+1) is emitted
    inside step s's act1 group (it writes the opposite PSUM bank of the
    2-deep rotation and its WAR gate act1(s-1) matches that group).
  - 3-deep rotation of SBUF state, 2-deep rotation of PSUM banks
    (3 windows x 2 + pgarb = 7 of 8 banks; pgarb doubles as FC target).
  - Init: states zeroed by VectorE memsets + one 32-row DMA seeding the
    ones chain; input DMAs split across the sync and gpsimd DGE queues
    (a DMA occupies its queue ~600ns); a dummy 1-col tanh is emitted
    first so the ~1.3us ACT_TABLE_LOAD overlaps the input DMAs.
"""

import copy
import numpy as np

import concourse.bass as bass
import concourse.tile as tile
from concourse import mybir
from concourse import bass_utils

N_CORES = 8
B, T, D, H, L = 1024, 512, 16, 30, 10
BC = B // N_CORES          # batch per core = 128
RING = 8                   # resident x blocks
ACT_SPLIT = 3              # acts per step: 1, 2, or 3
DEPTH = 3                  # state rotation depth
N_FILL = 0                 # HAM-warming filler matmuls per step
N_WARM = 0                 # pipeline-warmer matmuls per step (<= ACT_SPLIT)

F16 = mybir.dt.float16
F32 = mybir.dt.float32

# wbuf column layout (all matmul weight blocks are 128 cols so every
# matmul is full 128x128 mode -- no PE tiling-mode switches)
WC_R0, WC_R1, WC_R2 = 0, 128, 256
WC_INJ = 384               # + 128*v, v=0..7
WC_DUP3 = WC_INJ + 8 * 128
WC_DUP6 = WC_DUP3 + 128
WC_FC = WC_DUP6 + 128
W_COLS = WC_FC + 32


def _decouple_ldweights(nc):
    """Emit LDWEIGHTS separately from MATMUL (walrus: ldweights=False).
    Decoupled loads pipeline with the previous matmul's streaming: measured
    56ns/MM vs 107ns self-loading for 128-col fp16 weights."""
    for fn in nc.m.functions:
        for blk in fn.blocks:
            for inst in blk.instructions:
                if isinstance(inst, mybir.InstMatmult):
                    inst.ldweights = False


def _hoist_matmul_waits(nc):
    """Move each matmul's sem waits onto a PE NoOp inserted right before it
    (i.e. after its LDWEIGHTS). Otherwise bacc's move_matmul_waits_to_
    ldweights pins the wait on the LDWEIGHTS, serializing the 105ns weight
    load behind the tanh on the critical path; with the NoOp the weight
    load runs during the previous step's tanh."""
    for fn in nc.m.functions:
        for blk in fn.blocks:
            newlist = []
            for inst in blk.instructions:
                si = inst.sync_info
                if (isinstance(inst, mybir.InstMatmult) and si is not None
                        and si.on_wait):
                    for j, w in enumerate(si.on_wait):
                        pre = mybir.InstNoOp(
                            name=f"{inst.name}_mmw{j}",
                            sync_info=mybir.SyncInfo(on_wait=[w], on_update=[]),
                            bass_nofuse=True,
                            engine=inst.engine,
                        )
                        nc.register_instruction(pre, overwrite=True)
                        newlist.append(pre)
                    inst.sync_info = copy.replace(si, on_wait=[])
                newlist.append(inst)
            blk.instructions = newlist


def _strip_same_engine_waits(nc):
    """Drop waits on semaphores whose only updaters are compute
    instructions on the SAME engine as the waiter. Same-engine streams
    issue AND complete in order (PE matmuls are pc-monotone, ACT/DVE are
    single in-order pipelines), so these WAW/ordering waits are redundant
    -- and each one otherwise becomes a hoisted NoOp costing ScalarE NX
    issue slots (~8ns/act = ~12us over the run). DMA-completion sems are
    hardware-updated out of queue order and are never stripped."""
    hw_async = ("InstDMACopy", "InstTensorLoad")
    upd = {}
    for fn in nc.m.functions:
        for blk in fn.blocks:
            for inst in blk.instructions:
                si = getattr(inst, "sync_info", None)
                if si is not None and si.on_update:
                    for u in si.on_update:
                        upd.setdefault(u.id, set()).add(
                            (type(inst).__name__, inst.engine))

    def strippable(w, eng):
        ups = upd.get(w.id)
        if not ups:
            return False
        return all(tn not in hw_async and e == eng for tn, e in ups)

    for fn in nc.m.functions:
        for blk in fn.blocks:
            newlist = []
            for inst in blk.instructions:
                si = getattr(inst, "sync_info", None)
                if si is not None and si.on_wait:
                    keep = [w for w in si.on_wait
                            if not strippable(w, inst.engine)]
                    if len(keep) != len(si.on_wait):
                        inst.sync_info = copy.replace(si, on_wait=keep)
                        si = inst.sync_info
                if (isinstance(inst, mybir.InstNoOp) and si is not None
                        and not si.on_wait and not si.on_update):
                    continue  # wait fully stripped: drop the NoOp
                newlist.append(inst)
            blk.instructions = newlist


def _split_sync_waits(nc, limit=1):
    """walrus CoreV2/V3 lowering rejects instructions whose sync_info carries
    more than ~1 wait condition. Hoist excess waits onto same-engine NoOps
    inserted immediately before the offending instruction (engines execute
    their stream in order, so the waits still gate it)."""
    for fn in nc.m.functions:
        for blk in fn.blocks:
            newlist = []
            for inst in blk.instructions:
                si = inst.sync_info
                if si is not None and si.on_wait and len(si.on_wait) > limit:
                    waits = list(si.on_wait)
                    extra, keep = waits[:-limit], waits[-limit:]
                    for j, w in enumerate(extra):
                        pre = mybir.InstNoOp(
                            name=f"{inst.name}_w{j}",
                            sync_info=mybir.SyncInfo(on_wait=[w], on_update=[]),
                            bass_nofuse=True,
                            engine=inst.engine,
                        )
                        nc.register_instruction(pre, overwrite=True)
                        newlist.append(pre)
                    inst.sync_info = copy.replace(si, on_wait=keep)
                newlist.append(inst)
            blk.instructions = newlist


def build_kernel(t_steps=T):
    nblk = (t_steps + 7) // 8
    xt_blocks = nblk + 8
    n_steps = t_steps + L - 1  # wavefront steps

    nc = bass.Bass(trn_type="TRN2")
    xt = nc.dram_tensor("xt", [xt_blocks * 128, BC], F16, kind="ExternalInput")
    wbuf = nc.dram_tensor("wbuf", [128, W_COLS], F16, kind="ExternalInput")
    ones = nc.dram_tensor("ones", [32, BC], F16, kind="ExternalInput")
    y = nc.dram_tensor("y", [1, BC], F32, kind="ExternalOutput")

    with tile.TileContext(nc) as tc:
        with (
            tc.tile_pool(name="persist", bufs=1) as pp,
            tc.tile_pool(name="psum", bufs=1, space="PSUM") as pq,
        ):
            wt = pp.tile([128, W_COLS], F16, tag="wt", name="wt")
            ring = [pp.tile([128, BC], F16, tag=f"ring{i}", name=f"ring{i}")
                    for i in range(RING)]
            # one tile PER region window: tile-granular dependency tracking
            # must not couple the three act groups (a shared [128, 2*BC]
            # tile gives act1 a false RAW dep on R1 and inject a false WAR
            # on act2, serializing the whole step)
            saA = [pp.tile([128, BC], F16, tag=f"saA{i}", name=f"saA{i}")
                   for i in range(DEPTH)]
            saB = [pp.tile([128, BC], F16, tag=f"saB{i}", name=f"saB{i}")
                   for i in range(DEPTH)]
            sb = [pp.tile([128, BC], F16, tag=f"sb{i}", name=f"sb{i}")
                  for i in range(DEPTH)]
            # PSUM is 8 banks of 2KB/partition, one bank per tile: 2-deep
            # rotation x 3 windows + pgarb (warmer target, reused for the
            # final FC) = 7 banks. The WAR dep inject(s+2) <- act1(s) is
            # exactly the 2-deep reuse condition.
            PDEPTH = 2
            paA = [pq.tile([128, BC], F32, tag=f"paA{i}", name=f"paA{i}")
                   for i in range(PDEPTH)]
            paB = [pq.tile([128, BC], F32, tag=f"paB{i}", name=f"paB{i}")
                   for i in range(PDEPTH)]
            pb = [pq.tile([128, BC], F32, tag=f"pb{i}", name=f"pb{i}")
                  for i in range(PDEPTH)]
            pgarb = pq.tile([128, BC], F32, tag="pgarb", name="pgarb")
            gact = pp.tile([1, 1], F16, tag="gact", name="gact")

            # --- init ---
            # dummy tanh first: walrus places the ~1.3us ACT_TABLE_LOAD
            # before the first ACTIVATE in the Scalar stream; with no data
            # deps this runs during the input DMAs instead of after them.
            nc.vector.memset(pgarb[:, :], 0.0)
            # seed buffer first: the critical path to the first act is
            # memset(saA[-1]) -> ones DMA -> R0(0); the ones DMA rides the
            # otherwise-idle scalar DGE queue (sync/gpsimd are busy with the
            # ring), and the dummy act below also pulls the ~1.3us
            # ACT_TABLE_LOAD (placed before the first ACTIVATE in the
            # scalar stream) under the input DMAs.
            seed = saA[(0 - 1) % DEPTH]
            nc.vector.memset(seed[:, :], 0.0)
            nc.scalar.dma_start(out=seed[0:32, :], in_=ones[:, :])
            nc.scalar.activation(gact[0:1, 0:1], pgarb[0:1, 0:1],
                                 mybir.ActivationFunctionType.Tanh)
            # input DMAs spread across DGE queues (a DMA occupies its
            # issuing queue ~600ns; serialized on one queue they cost ~9us).
            # wt is split so the columns the first steps need (region
            # matrices + first inject blocks) land before the rest of the
            # 400KB weight image.
            nc.sync.dma_start(out=wt[:, 0:768], in_=wbuf[:, 0:768])
            nc.sync.dma_start(out=wt[:, 768:W_COLS], in_=wbuf[:, 768:W_COLS])
            qs = [nc.sync, nc.gpsimd]
            for i in range(RING):
                qs[i % 2].dma_start(out=ring[i][:, :],
                                    in_=xt[i * 128:(i + 1) * 128, :])
            for p_ in paA + paB + pb:
                nc.vector.memset(p_[:, :], 0.0)
            # state init: all zeros; the per-slot ones rows form a delay
            # chain (slot j's ones regenerates from slot j-1's ones), so only
            # W1-slot0's ones (partition 30 of the step-(-1) buffer) is
            # seeded; every other slot turns on exactly when its layer
            # becomes active and inactive slots stay exactly 0 -- no warmup
            # zeroing needed. (SBUF APs need 32-aligned partition bases, so
            # the single row comes via a 32-row DMA with 1.0 at row 30.)
            for s_ in saA + saB + sb:
                if s_ is not seed:
                    nc.vector.memset(s_[:, :], 0.0)

            def emit_inject(t):
                # x_t @ W_ih0^T into paA[t%3]; no tanh dependency
                blk = (t // 8) % RING
                nc.tensor.matmul(paA[t % PDEPTH][0:128, 0:BC],
                                 wt[:, WC_INJ + 128 * (t % 8):
                                    WC_INJ + 128 * (t % 8) + 128],
                                 ring[blk][:, :], start=True, stop=False,
                                 skip_group_check=True)

            def emit_step(s):
                j = (s - 1) % DEPTH  # state buffers of step s-1
                k = s % DEPTH
                kp = s % PDEPTH
                r0 = saA[j][:, :]
                r1 = saB[j][:, :]
                r2 = sb[j][:, :]

                if s % 8 == 0:
                    b = s // 8 + 4
                    # blocks 0..RING-1 were loaded at init; don't reload
                    if RING <= b < xt_blocks:
                        nc.sync.dma_start(out=ring[b % RING][:, :],
                                          in_=xt[b * 128:(b + 1) * 128, :])

                # Window liveness: W2's first nonzero output (h3') is at
                # s=3 and W3's (h6') at s=6; before that every window-2/3
                # slot is exactly 0, which the zero-initialized state
                # rotation already provides -- so those groups are simply
                # not emitted. At the tail, W1 is last needed at s=T+2
                # (h3 for dup3), dup3 at s=T+2 (h3' for layer 4 at T+3),
                # R1/act2 at s=T+4 (h5 for dup6), dup6 at s=T+5 (h6' for
                # layer 7 at T+6). Later garbage feeds only t>=T cells that
                # never reach h9(T-1).
                w1_on = s <= t_steps + 2
                dup3_on = 3 <= s <= t_steps + 2
                r1_on = 3 <= s <= t_steps + 4
                dup6_on = 6 <= s <= t_steps + 5
                w3_on = s >= 6
                inj = s < t_steps
                # The PE executes its stream IN ORDER, so matmuls must be
                # emitted monotone in their gating act: a matmul gated on a
                # late act would block every later matmul in the stream.
                # --- gated on act1(s-1): R0 merged + dup3 (reads r0) ---
                if w1_on:
                    nc.tensor.matmul(paA[kp][:, :], wt[:, WC_R0:WC_R0 + 128],
                                     r0, start=not inj, stop=True,
                                     skip_group_check=True)
                if dup3_on:
                    nc.tensor.matmul(paB[kp][:, :],
                                     wt[:, WC_DUP3:WC_DUP3 + 128],
                                     r0, start=True, stop=False,
                                     skip_group_check=True)
                # inject(s+1) pulled 1 step ahead: writes paA[(s+1)%2] (the
                # opposite bank), WAR-gated on act1(s-1) like this group, so
                # it never sits behind step s+1's gated matmuls.
                if s + 1 < t_steps:
                    emit_inject(s + 1)
                for _f in range(N_FILL):
                    nc.tensor.matmul(pgarb[:, :],
                                     wt[:, WC_R0:WC_R0 + 128],
                                     ring[(s + _f) % RING][:, :],
                                     start=True, stop=True,
                                     skip_group_check=True)
                # --- gated on act2(s-1): R1 merged + dup6 (read r1) ---
                if r1_on:
                    nc.tensor.matmul(paB[kp][:, :],
                                     wt[:, WC_R1:WC_R1 + 128],
                                     r1, start=not dup3_on, stop=True,
                                     skip_group_check=True)
                if dup6_on:
                    nc.tensor.matmul(pb[kp][0:128, 0:BC],
                                     wt[:, WC_DUP6:WC_DUP6 + 128],
                                     r1, start=True, stop=False,
                                     skip_group_check=True)
                # --- gated on act3(s-1): R2 merged (reads r2) ---
                if w3_on:
                    nc.tensor.matmul(pb[kp][0:128, 0:BC],
                                     wt[:, WC_R2:WC_R2 + 128],
                                     r2, start=not dup6_on, stop=True,
                                     skip_group_check=True)

                # ---- activations (one per live region window) ----
                if w1_on:
                    nc.scalar.activation(saA[k][:, :], paA[kp][:, :],
                                         mybir.ActivationFunctionType.Tanh)
                if r1_on or dup3_on:
                    nc.scalar.activation(saB[k][:, :], paB[kp][:, :],
                                         mybir.ActivationFunctionType.Tanh)
                if w3_on:
                    nc.scalar.activation(sb[k][:, :], pb[kp][:, :],
                                         mybir.ActivationFunctionType.Tanh)

            # prologue inject for step 0 (steady-state steps emit inject(s+1))
            if t_steps > 0:
                emit_inject(0)
            for s in range(n_steps):
                emit_step(s)

            # ---- FC on h9 of last timestep (in sb[(n_steps-1)%DEPTH] slot 3)
            fin = sb[(n_steps - 1) % DEPTH][:, :]
            nc.tensor.matmul(pgarb[0:1, :], wt[:, WC_FC:WC_FC + 1], fin,
                             start=True, stop=True)
            yout = pp.tile([1, BC], F32, tag="yout", name="yout")
            nc.vector.tensor_copy(yout[0:1, :], pgarb[0:1, :])
            nc.sync.dma_start(out=y[:, :], in_=yout[0:1, :])

    _decouple_ldweights(nc)
    _strip_same_engine_waits(nc)
    _split_sync_waits(nc)
    return nc


def prep_core_inputs(x_core, W_ih0, W_ih, W_hh, b_ih, b_hh, fc_w, fc_b,
                     t_steps=T):
    """Host-side marshaling for one core. x_core: [BC, t_steps, D] fp32."""
    nblk = (t_steps + 7) // 8
    xt_blocks = nblk + 8
    # XT[g*128 + 16*(t%8)+d, b] = x[b, 8g + t%8, d]
    xt = np.zeros((xt_blocks * 128, BC), np.float16)
    xr = np.transpose(x_core, (1, 2, 0))  # [t, d, b]
    tpad = nblk * 8
    if t_steps != tpad:
        xr = np.concatenate([xr, np.zeros((tpad - t_steps, D, BC), xr.dtype)],
                            0)
    xt[:nblk * 128, :] = xr.reshape(nblk * 128, BC)

    wbuf = np.zeros((128, W_COLS), np.float32)

    # Per-slot ones rows: slot j's ones lives at row 32j+30 of its window.
    # Bias for slot j rides on its own (recurrent-input) ones row; each
    # slot's ones regenerates from the previous slot's ones (30.0 -> tanh
    # -> 1.0 delay chain), so ones_l turns on exactly after step l-1 and
    # inactive slots stay exactly 0 during the wavefront warmup.
    def put_region(col0, layers, chain):
        # layers: (out_slot, feed_slot_or_None, Wi_or_None, rec_slot, Wh, b)
        for out_slot, fslot, Wi, rslot, Wh, bias in layers:
            c = col0 + 32 * out_slot
            if Wi is not None:
                wbuf[32 * fslot:32 * fslot + Wi.shape[1], c:c + 30] = Wi.T
            wbuf[32 * rslot:32 * rslot + 30, c:c + 30] = Wh.T
            wbuf[32 * out_slot + 30, c:c + 30] = bias
        for src_row, out_slot in chain:
            wbuf[src_row, col0 + 32 * out_slot + 30] = 30.0

    bias = b_ih + b_hh
    put_region(WC_R0, [
        (0, None, None, 0, W_hh[0], bias[0]),
        (1, 0, W_ih[0], 1, W_hh[1], bias[1]),
        (2, 1, W_ih[1], 2, W_hh[2], bias[2]),
        (3, 2, W_ih[2], 3, W_hh[3], bias[3]),
    ], chain=[(30, 0), (30, 1), (62, 2), (94, 3)])
    # W2/W3 slot0 AND slot1 ones both regenerate through the dup matmul
    # (from the previous window's slot-3 ones) so layer 3k+1's bias is on
    # one step earlier than the in-window chain could deliver it.
    put_region(WC_R1, [
        (1, 0, W_ih[3], 1, W_hh[4], bias[4]),
        (2, 1, W_ih[4], 2, W_hh[5], bias[5]),
        (3, 2, W_ih[5], 3, W_hh[6], bias[6]),
    ], chain=[(62, 2), (94, 3)])
    # W3's row126 additionally regenerates from ITSELF: the rest of the W3
    # chain dies in the tail once dup6 stops (its feeders are skipped), and
    # row126 must still be 1 at the FC step to deliver fc_b and bias9.
    put_region(WC_R2, [
        (1, 0, W_ih[6], 1, W_hh[7], bias[7]),
        (2, 1, W_ih[7], 2, W_hh[8], bias[8]),
        (3, 2, W_ih[8], 3, W_hh[9], bias[9]),
    ], chain=[(62, 2), (94, 3), (126, 3)])
    for v in range(8):
        wbuf[16 * v:16 * v + 16, WC_INJ + 128 * v:WC_INJ + 128 * v + 30] = \
            W_ih0.T
    # dup3 = layer-3 columns of R0 matrix (bias3 rides along on row 126);
    # dup6 = layer-6 columns of R1
    wbuf[:, WC_DUP3:WC_DUP3 + 30] = wbuf[:, WC_R0 + 96:WC_R0 + 126]
    wbuf[126, WC_DUP3 + 30] = 30.0   # W2-slot0 ones <- W1-slot3 ones
    wbuf[126, WC_DUP3 + 62] = 30.0   # W2-slot1 ones <- W1-slot3 ones
    wbuf[:, WC_DUP6:WC_DUP6 + 30] = wbuf[:, WC_R1 + 96:WC_R1 + 126]
    wbuf[126, WC_DUP6 + 30] = 30.0   # W3-slot0 ones <- W2-slot3 ones
    wbuf[126, WC_DUP6 + 62] = 30.0   # W3-slot1 ones <- W2-slot3 ones
    wbuf[96:126, WC_FC] = fc_w[0]
    wbuf[126, WC_FC] = fc_b[0]

    ones = np.zeros((32, BC), np.float16)
    ones[30, :] = 1.0  # seeds W1-slot0's ones (partition 30)
    return {"xt": xt, "wbuf": wbuf.astype(np.float16), "ones": ones}


_CACHE = {}


def run(x, W_ih0, W_ih, W_hh, b_ih, b_hh, fc_w, fc_b, t_steps=T):
    x = np.asarray(x, np.float32)
    args = [np.asarray(a, np.float32) for a in
            (W_ih0, W_ih, W_hh, b_ih, b_hh, fc_w, fc_b)]
    key = t_steps
    if key not in _CACHE:
        _CACHE[key] = build_kernel(t_steps)
    nc = _CACHE[key]
    in_maps = [prep_core_inputs(x[c * BC:(c + 1) * BC], *args, t_steps=t_steps)
               for c in range(N_CORES)]
    res = bass_utils.run_bass_kernel_spmd(nc, in_maps,
                                          core_ids=list(range(N_CORES)))
    out = np.concatenate([res.results[c]["y"].reshape(BC, 1)
                          for c in range(N_CORES)], axis=0)
    return out, res


def kernel(x, W_ih0, W_ih, W_hh, b_ih, b_hh, fc_w, fc_b):
    out, _ = run(x, W_ih0, W_ih, W_hh, b_ih, b_hh, fc_w, fc_b)
    return out


if __name__ == "__main__":
    t_small = 32
    rng = np.random.default_rng(0)
    s = 1.0 / np.sqrt(H)
    x = rng.standard_normal((B, t_small, D)).astype(np.float32)
    W_ih0 = (rng.standard_normal((H, D)) * s).astype(np.float32)
    W_ih = (rng.standard_normal((L - 1, H, H)) * s).astype(np.float32)
    W_hh = (rng.standard_normal((L, H, H)) * s).astype(np.float32)
    b_ih = (rng.standard_normal((L, H)) * s).astype(np.float32)
    b_hh = (rng.standard_normal((L, H)) * s).astype(np.float32)
    fc_w = (rng.standard_normal((1, H)) * s).astype(np.float32)
    fc_b = (rng.standard_normal((1,)) * s).astype(np.float32)

    def ref_np(x):
        out = x
        for l in range(L):
            Wi = W_ih0 if l == 0 else W_ih[l - 1]
            xw = np.einsum("btd,hd->bth", out, Wi) + (b_ih[l] + b_hh[l])
            h = np.zeros((x.shape[0], H), np.float32)
            ys = np.empty((x.shape[0], xw.shape[1], H), np.float32)
            for t in range(xw.shape[1]):
                h = np.tanh(xw[:, t] + h @ W_hh[l].T)
                ys[:, t] = h
            out = ys
        return out[:, -1, :] @ fc_w.T + fc_b

    want = ref_np(x)
    got, _ = run(x, W_ih0, W_ih, W_hh, b_ih, b_hh, fc_w, fc_b, t_steps=t_small)
    err = np.abs(got - want).max() / (np.abs(want).max() + 1e-9)
    print("small-T rel err:", err)


# revision 36
# speedup vs baseline: 1.0003x; 1.0003x over previous
"""ComplexRNN Trainium2 kernel.

10-layer tanh RNN, B=1024, T=512, D=16, H=30, final FC on last timestep.

Strategy (per core, 8-way batch-parallel, 128 batch rows/core):
  - Hidden-major layout: state h^l lives in SBUF as [30 partitions, 128 batch].
  - Layer wavefront: at step s, layer l computes timestep t = s - l. All
    10 layers advance each step; all dependencies are on step s-1.
  - States packed into 3 "region" windows of 128 partitions (4 slots of 32):
      W1 = [h0 h1 h2 h3], W2 = [h3' h4 h5 h6], W3 = [h6' h7 h8 h9]
    (h3', h6' are duplicates written by extra matmuls so each layer finds
    its feed + recurrent state inside one 128-partition window).
  - ONE fp16 matmul per region per step (K=128, M=128, N=128): a block
    matrix computes all of the region's layer updates at once. Plus one
    "inject" matmul for x_t @ W_ih0^T (reading a host-pre-transposed x
    ring) and two duplicate matmuls (dup3 -> h3', dup6 -> h6'), whose
    weights are column slices of the merged region matrices.
  - Per-slot ones rows: slot j's ones lives at partition 32j+30; its
    bias rides on that row, and each ones regenerates from the previous
    slot's ones (weight 30.0 -> tanh -> 1.0), forming a delay chain
    seeded only at W1-slot0. A slot's ones (and hence its bias) turns on
    exactly when its layer becomes active, so inactive slots stay
    exactly 0 through the wavefront warmup: no warmup zeroing at all.
  - tanh: ACTIVATE (PSUM -> SBUF fp16); ScalarE is the critical engine.
    THREE acts per step (one per 128-col region window). With 2 acts the
    binding cycle is act(473ns) + its 3-matmul dependency chain (442ns)
    = 915ns/step; with 3 acts each act gates only 1-2 matmuls (chain
    ~390ns hides under the other two acts) and the period drops to the
    ScalarE issue floor ~3*258 = 775ns/step.
  - Each window has its OWN SBUF state tile and PSUM tile: dependency
    tracking is tile-granular, so a shared [128, 2*BC] tile gives act1 a
    false RAW dep on R1 and inject a false WAR on act2, serializing the
    step (and collapsing the PE to its cold clock).
  - The PE executes in order, so matmuls are emitted monotone in their
    gating act (R0,dup3 | R1,dup6 | R2) and inject(s+1) is emitted
    inside step s's act1 group (it writes the opposite PSUM bank of the
    2-deep rotation and its WAR gate act1(s-1) matches that group).
  - 3-deep rotation of SBUF state, 2-deep rotation of PSUM banks
    (3 windows x 2 + pgarb = 7 of 8 banks; pgarb doubles as FC target).
  - Init: states zeroed by VectorE memsets + one 32-row DMA seeding the
    ones chain; input DMAs split across the sync and gpsimd DGE queues
    (a DMA occupies its queue ~600ns); a dummy 1-col tanh is emitted
    first so the ~1.3us ACT_TABLE_LOAD overlaps the input DMAs.
"""

import copy
import numpy as np

import concourse.bass as bass
import concourse.tile as tile
from concourse import mybir
from concourse import bass_utils

N_CORES = 8
B, T, D, H, L = 1024, 512, 16, 30, 10
BC = B // N_CORES          # batch per core = 128
RING = 8                   # resident x blocks
ACT_SPLIT = 3              # acts per step: 1, 2, or 3
DEPTH = 3                  # state rotation depth
N_FILL = 0                 # HAM-warming filler matmuls per step
N_WARM = 0                 # pipeline-warmer matmuls per step (<= ACT_SPLIT)

F16 = mybir.dt.float16
F32 = mybir.dt.float32

# wbuf column layout (all matmul weight blocks are 128 cols so every
# matmul is full 128x128 mode -- no PE tiling-mode switches)
WC_R0, WC_R1, WC_R2 = 0, 128, 256
WC_INJ = 384               # + 128*v, v=0..7
WC_DUP3 = WC_INJ + 8 * 128
WC_DUP6 = WC_DUP3 + 128
WC_FC = WC_DUP6 + 128
W_COLS = WC_FC + 32


def _decouple_ldweights(nc):
    """Emit LDWEIGHTS separately from MATMUL (walrus: ldweights=False).
    Decoupled loads pipeline with the previous matmul's streaming: measured
    56ns/MM vs 107ns self-loading for 128-col fp16 weights."""
    for fn in nc.m.functions:
        for blk in fn.blocks:
            for inst in blk.instructions:
                if isinstance(inst, mybir.InstMatmult):
                    inst.ldweights = False


def _hoist_matmul_waits(nc):
    """Move each matmul's sem waits onto a PE NoOp inserted right before it
    (i.e. after its LDWEIGHTS). Otherwise bacc's move_matmul_waits_to_
    ldweights pins the wait on the LDWEIGHTS, serializing the 105ns weight
    load behind the tanh on the critical path; with the NoOp the weight
    load runs during the previous step's tanh."""
    for fn in nc.m.functions:
        for blk in fn.blocks:
            newlist = []
            for inst in blk.instructions:
                si = inst.sync_info
                if (isinstance(inst, mybir.InstMatmult) and si is not None
                        and si.on_wait):
                    for j, w in enumerate(si.on_wait):
                        pre = mybir.InstNoOp(
                            name=f"{inst.name}_mmw{j}",
                            sync_info=mybir.SyncInfo(on_wait=[w], on_update=[]),
                            bass_nofuse=True,
                            engine=inst.engine,
                        )
                        nc.register_instruction(pre, overwrite=True)
                        newlist.append(pre)
                    inst.sync_info = copy.replace(si, on_wait=[])
                newlist.append(inst)
            blk.instructions = newlist


def _strip_same_engine_waits(nc):
    """Drop waits on semaphores whose only updaters are compute
    instructions on the SAME engine as the waiter. Same-engine streams
    issue AND complete in order (PE matmuls are pc-monotone, ACT/DVE are
    single in-order pipelines), so these WAW/ordering waits are redundant
    -- and each one otherwise becomes a hoisted NoOp costing ScalarE NX
    issue slots (~8ns/act = ~12us over the run). DMA-completion sems are
    hardware-updated out of queue order and are never stripped."""
    hw_async = ("InstDMACopy", "InstTensorLoad")
    upd = {}
    for fn in nc.m.functions:
        for blk in fn.blocks:
            for inst in blk.instructions:
                si = getattr(inst, "sync_info", None)
                if si is not None and si.on_update:
                    for u in si.on_update:
                        upd.setdefault(u.id, set()).add(
                            (type(inst).__name__, inst.engine))

    def strippable(w, eng):
        ups = upd.get(w.id)
        if not ups:
            return False
        return all(tn not in hw_async and e == eng for tn, e in ups)

    for fn in nc.m.functions:
        for blk in fn.blocks:
            newlist = []
            for inst in blk.instructions:
                si = getattr(inst, "sync_info", None)
                if si is not None and si.on_wait:
                    keep = [w for w in si.on_wait
                            if not strippable(w, inst.engine)]
                    if len(keep) != len(si.on_wait):
                        inst.sync_info = copy.replace(si, on_wait=keep)
                        si = inst.sync_info
                if (isinstance(inst, mybir.InstNoOp) and si is not None
                        and not si.on_wait and not si.on_update):
                    continue  # wait fully stripped: drop the NoOp
                newlist.append(inst)
            blk.instructions = newlist


def _split_sync_waits(nc, limit=1):
    """walrus CoreV2/V3 lowering rejects instructions whose sync_info carries
    more than ~1 wait condition. Hoist excess waits onto same-engine NoOps
    inserted immediately before the offending instruction (engines execute
    their stream in order, so the waits still gate it)."""
    for fn in nc.m.functions:
        for blk in fn.blocks:
            newlist = []
            for inst in blk.instructions:
                si = inst.sync_info
                if si is not None and si.on_wait and len(si.on_wait) > limit:
                    waits = list(si.on_wait)
                    extra, keep = waits[:-limit], waits[-limit:]
                    for j, w in enumerate(extra):
                        pre = mybir.InstNoOp(
                            name=f"{inst.name}_w{j}",
                            sync_info=mybir.SyncInfo(on_wait=[w], on_update=[]),
                            bass_nofuse=True,
                            engine=inst.engine,
                        )
                        nc.register_instruction(pre, overwrite=True)
                        newlist.append(pre)
                    inst.sync_info = copy.replace(si, on_wait=keep)
                newlist.append(inst)
            blk.instructions = newlist


def build_kernel(t_steps=T):
    nblk = (t_steps + 7) // 8
    xt_blocks = nblk + 8
    n_steps = t_steps + L - 1  # wavefront steps

    nc = bass.Bass(trn_type="TRN2")
    xt = nc.dram_tensor("xt", [xt_blocks * 128, BC], F16, kind="ExternalInput")
    wbuf = nc.dram_tensor("wbuf", [128, W_COLS], F16, kind="ExternalInput")
    ones = nc.dram_tensor("ones", [32, BC], F16, kind="ExternalInput")
    y = nc.dram_tensor("y", [1, BC], F32, kind="ExternalOutput")

    with tile.TileContext(nc) as tc:
        with (
            tc.tile_pool(name="persist", bufs=1) as pp,
            tc.tile_pool(name="psum", bufs=1, space="PSUM") as pq,
        ):
            wt = pp.tile([128, W_COLS], F16, tag="wt", name="wt")
            ring = [pp.tile([128, BC], F16, tag=f"ring{i}", name=f"ring{i}")
                    for i in range(RING)]
            # one tile PER region window: tile-granular dependency tracking
            # must not couple the three act groups (a shared [128, 2*BC]
            # tile gives act1 a false RAW dep on R1 and inject a false WAR
            # on act2, serializing the whole step)
            saA = [pp.tile([128, BC], F16, tag=f"saA{i}", name=f"saA{i}")
                   for i in range(DEPTH)]
            saB = [pp.tile([128, BC], F16, tag=f"saB{i}", name=f"saB{i}")
                   for i in range(DEPTH)]
            sb = [pp.tile([128, BC], F16, tag=f"sb{i}", name=f"sb{i}")
                  for i in range(DEPTH)]
            # PSUM is 8 banks of 2KB/partition, one bank per tile: 2-deep
            # rotation x 3 windows + pgarb (warmer target, reused for the
            # final FC) = 7 banks. The WAR dep inject(s+2) <- act1(s) is
            # exactly the 2-deep reuse condition.
            PDEPTH = 2
            paA = [pq.tile([128, BC], F32, tag=f"paA{i}", name=f"paA{i}")
                   for i in range(PDEPTH)]
            paB = [pq.tile([128, BC], F32, tag=f"paB{i}", name=f"paB{i}")
                   for i in range(PDEPTH)]
            pb = [pq.tile([128, BC], F32, tag=f"pb{i}", name=f"pb{i}")
                  for i in range(PDEPTH)]
            pgarb = pq.tile([128, BC], F32, tag="pgarb", name="pgarb")
            gact = pp.tile([1, 1], F16, tag="gact", name="gact")

            # --- init ---
            # dummy tanh first: walrus places the ~1.3us ACT_TABLE_LOAD
            # before the first ACTIVATE in the Scalar stream; with no data
            # deps this runs during the input DMAs instead of after them.
            nc.vector.memset(pgarb[:, :], 0.0)
            # seed buffer first: the critical path to the first act is
            # memset(saA[-1]) -> ones DMA -> R0(0); the ones DMA rides the
            # otherwise-idle scalar DGE queue (sync/gpsimd are busy with the
            # ring), and the dummy act below also pulls the ~1.3us
            # ACT_TABLE_LOAD (placed before the first ACTIVATE in the
            # scalar stream) under the input DMAs.
            seed = saA[(0 - 1) % DEPTH]
            nc.vector.memset(seed[:, :], 0.0)
            nc.scalar.dma_start(out=seed[0:32, :], in_=ones[:, :])
            nc.scalar.activation(gact[0:1, 0:1], pgarb[0:1, 0:1],
                                 mybir.ActivationFunctionType.Tanh)
            # input DMAs spread across DGE queues (a DMA occupies its
            # issuing queue ~600ns; serialized on one queue they cost ~9us).
            # wt is split so the columns the first steps need (region
            # matrices + first inject blocks) land before the rest of the
            # 400KB weight image.
            nc.sync.dma_start(out=wt[:, 0:768], in_=wbuf[:, 0:768])
            nc.sync.dma_start(out=wt[:, 768:W_COLS], in_=wbuf[:, 768:W_COLS])
            qs = [nc.sync, nc.gpsimd]
            for i in range(RING):
                qs[i % 2].dma_start(out=ring[i][:, :],
                                    in_=xt[i * 128:(i + 1) * 128, :])
            for p_ in paA + paB + pb:
                nc.vector.memset(p_[:, :], 0.0)
            # state init: all zeros; the per-slot ones rows form a delay
            # chain (slot j's ones regenerates from slot j-1's ones), so only
            # W1-slot0's ones (partition 30 of the step-(-1) buffer) is
            # seeded; every other slot turns on exactly when its layer
            # becomes active and inactive slots stay exactly 0 -- no warmup
            # zeroing needed. (SBUF APs need 32-aligned partition bases, so
            # the single row comes via a 32-row DMA with 1.0 at row 30.)
            for s_ in saA + saB + sb:
                if s_ is not seed:
                    nc.vector.memset(s_[:, :], 0.0)

            def emit_inject(t):
                # x_t @ W_ih0^T into paA[t%3]; no tanh dependency
                blk = (t // 8) % RING
                nc.tensor.matmul(paA[t % PDEPTH][0:128, 0:BC],
                                 wt[:, WC_INJ + 128 * (t % 8):
                                    WC_INJ + 128 * (t % 8) + 128],
                                 ring[blk][:, :], start=True, stop=False,
                                 skip_group_check=True)

            def emit_step(s):
                j = (s - 1) % DEPTH  # state buffers of step s-1
                k = s % DEPTH
                kp = s % PDEPTH
                r0 = saA[j][:, :]
                r1 = saB[j][:, :]
                r2 = sb[j][:, :]

                if s % 8 == 0:
                    b = s // 8 + 4
                    # blocks 0..RING-1 were loaded at init; don't reload
                    if RING <= b < xt_blocks:
                        nc.sync.dma_start(out=ring[b % RING][:, :],
                                          in_=xt[b * 128:(b + 1) * 128, :])

                # Window liveness: W2's first nonzero output (h3') is at
                # s=3 and W3's (h6') at s=6; before that every window-2/3
                # slot is exactly 0, which the zero-initialized state
                # rotation already provides -- so those groups are simply
                # not emitted. At the tail, W1 is last needed at s=T+2
                # (h3 for dup3), dup3 at s=T+2 (h3' for layer 4 at T+3),
                # R1/act2 at s=T+4 (h5 for dup6), dup6 at s=T+5 (h6' for
                # layer 7 at T+6). Later garbage feeds only t>=T cells that
                # never reach h9(T-1).
                w1_on = s <= t_steps + 2
                dup3_on = 3 <= s <= t_steps + 2
                r1_on = 3 <= s <= t_steps + 4
                dup6_on = 6 <= s <= t_steps + 5
                w3_on = s >= 6
                inj = s < t_steps
                # The PE executes its stream IN ORDER, so matmuls must be
                # emitted monotone in their gating act: a matmul gated on a
                # late act would block every later matmul in the stream.
                # --- gated on act1(s-1): R0 merged + dup3 (reads r0) ---
                if w1_on:
                    nc.tensor.matmul(paA[kp][:, :], wt[:, WC_R0:WC_R0 + 128],
                                     r0, start=not inj, stop=True,
                                     skip_group_check=True)
                if dup3_on:
                    nc.tensor.matmul(paB[kp][:, :],
                                     wt[:, WC_DUP3:WC_DUP3 + 128],
                                     r0, start=True, stop=False,
                                     skip_group_check=True)
                # inject(s+1) pulled 1 step ahead: writes paA[(s+1)%2] (the
                # opposite bank), WAR-gated on act1(s-1) like this group, so
                # it never sits behind step s+1's gated matmuls.
                if s + 1 < t_steps:
                    emit_inject(s + 1)
                for _f in range(N_FILL):
                    nc.tensor.matmul(pgarb[:, :],
                                     wt[:, WC_R0:WC_R0 + 128],
                                     ring[(s + _f) % RING][:, :],
                                     start=True, stop=True,
                                     skip_group_check=True)
                # --- gated on act2(s-1): R1 merged + dup6 (read r1) ---
                if r1_on:
                    nc.tensor.matmul(paB[kp][:, :],
                                     wt[:, WC_R1:WC_R1 + 128],
                                     r1, start=not dup3_on, stop=True,
                                     skip_group_check=True)
                if dup6_on:
                    nc.tensor.matmul(pb[kp][0:128, 0:BC],
                                     wt[:, WC_DUP6:WC_DUP6 + 128],
                                     r1, start=True, stop=False,
                                     skip_group_check=True)
                # --- gated on act3(s-1): R2 merged (reads r2) ---
                if w3_on:
                    nc.tensor.matmul(pb[kp][0:128, 0:BC],
                                     wt[:, WC_R2:WC_R2 + 128],
                                     r2, start=not dup6_on, stop=True,
                                     skip_group_check=True)

                # ---- activations (one per live region window) ----
                if w1_on:
                    nc.scalar.activation(saA[k][:, :], paA[kp][:, :],
                                         mybir.ActivationFunctionType.Tanh)
                if r1_on or dup3_on:
                    nc.scalar.activation(saB[k][:, :], paB[kp][:, :],
                                         mybir.ActivationFunctionType.Tanh)
                if w3_on:
                    nc.scalar.activation(sb[k][:, :], pb[kp][:, :],
                                         mybir.ActivationFunctionType.Tanh)

            # prologue inject for step 0 (steady-state steps emit inject(s+1))
            if t_steps > 0:
                emit_inject(0)
            for s in range(n_steps):
                emit_step(s)

            # ---- FC on h9 of last timestep (in sb[(n_steps-1)%DEPTH] slot 3)
            fin = sb[(n_steps - 1) % DEPTH][:, :]
            nc.tensor.matmul(pgarb[0:1, :], wt[:, WC_FC:WC_FC + 1], fin,
                             start=True, stop=True)
            yout = pp.tile([1, BC], F32, tag="yout", name="yout")
            nc.vector.tensor_copy(yout[0:1, :], pgarb[0:1, :])
            nc.sync.dma_start(out=y[:, :], in_=yout[0:1, :])

    _decouple_ldweights(nc)
    # _strip_same_engine_waits is semantically safe but measured neutral-to-
    # slightly-negative (~+1us): the hoisted same-engine NoOp waits process
    # off the NX critical path, so stripping them buys nothing.
    _split_sync_waits(nc)
    return nc


def prep_core_inputs(x_core, W_ih0, W_ih, W_hh, b_ih, b_hh, fc_w, fc_b,
                     t_steps=T):
    """Host-side marshaling for one core. x_core: [BC, t_steps, D] fp32."""
    nblk = (t_steps + 7) // 8
    xt_blocks = nblk + 8
    # XT[g*128 + 16*(t%8)+d, b] = x[b, 8g + t%8, d]
    xt = np.zeros((xt_blocks * 128, BC), np.float16)
    xr = np.transpose(x_core, (1, 2, 0))  # [t, d, b]
    tpad = nblk * 8
    if t_steps != tpad:
        xr = np.concatenate([xr, np.zeros((tpad - t_steps, D, BC), xr.dtype)],
                            0)
    xt[:nblk * 128, :] = xr.reshape(nblk * 128, BC)

    wbuf = np.zeros((128, W_COLS), np.float32)

    # Per-slot ones rows: slot j's ones lives at row 32j+30 of its window.
    # Bias for slot j rides on its own (recurrent-input) ones row; each
    # slot's ones regenerates from the previous slot's ones (30.0 -> tanh
    # -> 1.0 delay chain), so ones_l turns on exactly after step l-1 and
    # inactive slots stay exactly 0 during the wavefront warmup.
    def put_region(col0, layers, chain):
        # layers: (out_slot, feed_slot_or_None, Wi_or_None, rec_slot, Wh, b)
        for out_slot, fslot, Wi, rslot, Wh, bias in layers:
            c = col0 + 32 * out_slot
            if Wi is not None:
                wbuf[32 * fslot:32 * fslot + Wi.shape[1], c:c + 30] = Wi.T
            wbuf[32 * rslot:32 * rslot + 30, c:c + 30] = Wh.T
            wbuf[32 * out_slot + 30, c:c + 30] = bias
        for src_row, out_slot in chain:
            wbuf[src_row, col0 + 32 * out_slot + 30] = 30.0

    bias = b_ih + b_hh
    put_region(WC_R0, [
        (0, None, None, 0, W_hh[0], bias[0]),
        (1, 0, W_ih[0], 1, W_hh[1], bias[1]),
        (2, 1, W_ih[1], 2, W_hh[2], bias[2]),
        (3, 2, W_ih[2], 3, W_hh[3], bias[3]),
    ], chain=[(30, 0), (30, 1), (62, 2), (94, 3)])
    # W2/W3 slot0 AND slot1 ones both regenerate through the dup matmul
    # (from the previous window's slot-3 ones) so layer 3k+1's bias is on
    # one step earlier than the in-window chain could deliver it.
    put_region(WC_R1, [
        (1, 0, W_ih[3], 1, W_hh[4], bias[4]),
        (2, 1, W_ih[4], 2, W_hh[5], bias[5]),
        (3, 2, W_ih[5], 3, W_hh[6], bias[6]),
    ], chain=[(62, 2), (94, 3)])
    # W3's row126 additionally regenerates from ITSELF: the rest of the W3
    # chain dies in the tail once dup6 stops (its feeders are skipped), and
    # row126 must still be 1 at the FC step to deliver fc_b and bias9.
    put_region(WC_R2, [
        (1, 0, W_ih[6], 1, W_hh[7], bias[7]),
        (2, 1, W_ih[7], 2, W_hh[8], bias[8]),
        (3, 2, W_ih[8], 3, W_hh[9], bias[9]),
    ], chain=[(62, 2), (94, 3), (126, 3)])
    for v in range(8):
        wbuf[16 * v:16 * v + 16, WC_INJ + 128 * v:WC_INJ + 128 * v + 30] = \
            W_ih0.T
    # dup3 = layer-3 columns of R0 matrix (bias3 rides along on row 126);
    # dup6 = layer-6 columns of R1
    wbuf[:, WC_DUP3:WC_DUP3 + 30] = wbuf[:, WC_R0 + 96:WC_R0 + 126]
    wbuf[126, WC_DUP3 + 30] = 30.0   # W2-slot0 ones <- W1-slot3 ones
    wbuf[126, WC_DUP3 + 62] = 30.0   # W2-slot1 ones <- W1-slot3 ones
    wbuf[:, WC_DUP6:WC_DUP6 + 30] = wbuf[:, WC_R1 + 96:WC_R1 + 126]
    wbuf[126, WC_DUP6 + 30] = 30.0   # W3-slot0 ones <- W2-slot3 ones
    wbuf[126, WC_DUP6 + 62] = 30.0   # W3-slot1 ones <- W2-slot3 ones
    wbuf[96:126, WC_FC] = fc_w[0]
    wbuf[126, WC_FC] = fc_b[0]

    ones = np.zeros((32, BC), np.float16)
    ones[30, :] = 1.0  # seeds W1-slot0's ones (partition 30)
    return {"xt": xt, "wbuf": wbuf.astype(np.float16), "ones": ones}


_CACHE = {}


def run(x, W_ih0, W_ih, W_hh, b_ih, b_hh, fc_w, fc_b, t_steps=T):
    x = np.asarray(x, np.float32)
    args = [np.asarray(a, np.float32) for a in
            (W_ih0, W_ih, W_hh, b_ih, b_hh, fc_w, fc_b)]
    key = t_steps
    if key not in _CACHE:
        _CACHE[key] = build_kernel(t_steps)
    nc = _CACHE[key]
    in_maps = [prep_core_inputs(x[c * BC:(c + 1) * BC], *args, t_steps=t_steps)
               for c in range(N_CORES)]
    res = bass_utils.run_bass_kernel_spmd(nc, in_maps,
                                          core_ids=list(range(N_CORES)))
    out = np.concatenate([res.results[c]["y"].reshape(BC, 1)
                          for c in range(N_CORES)], axis=0)
    return out, res


def kernel(x, W_ih0, W_ih, W_hh, b_ih, b_hh, fc_w, fc_b):
    out, _ = run(x, W_ih0, W_ih, W_hh, b_ih, b_hh, fc_w, fc_b)
    return out


if __name__ == "__main__":
    t_small = 32
    rng = np.random.default_rng(0)
    s = 1.0 / np.sqrt(H)
    x = rng.standard_normal((B, t_small, D)).astype(np.float32)
    W_ih0 = (rng.standard_normal((H, D)) * s).astype(np.float32)
    W_ih = (rng.standard_normal((L - 1, H, H)) * s).astype(np.float32)
    W_hh = (rng.standard_normal((L, H, H)) * s).astype(np.float32)
    b_ih = (rng.standard_normal((L, H)) * s).astype(np.float32)
    b_hh = (rng.standard_normal((L, H)) * s).astype(np.float32)
    fc_w = (rng.standard_normal((1, H)) * s).astype(np.float32)
    fc_b = (rng.standard_normal((1,)) * s).astype(np.float32)

    def ref_np(x):
        out = x
        for l in range(L):
            Wi = W_ih0 if l == 0 else W_ih[l - 1]
            xw = np.einsum("btd,hd->bth", out, Wi) + (b_ih[l] + b_hh[l])
            h = np.zeros((x.shape[0], H), np.float32)
            ys = np.empty((x.shape[0], xw.shape[1], H), np.float32)
            for t in range(xw.shape[1]):
                h = np.tanh(xw[:, t] + h @ W_hh[l].T)
                ys[:, t] = h
            out = ys
        return out[:, -1, :] @ fc_w.T + fc_b

    want = ref_np(x)
    got, _ = run(x, W_ih0, W_ih, W_hh, b_ih, b_hh, fc_w, fc_b, t_steps=t_small)
    err = np.abs(got - want).max() / (np.abs(want).max() + 1e-9)
    print("small-T rel err:", err)
